# revision 1
# baseline (speedup 1.0000x reference)
"""AttentionMIL Trainium2 kernel.

Math (per bag of 512 instances):
    emb    = relu(x @ w_enc + b_enc)            [512, 128]
    a      = tanh(emb @ w_att + b_att)          [512, 64]
    logits = a @ w_score (+ b_score, dropped: softmax shift-invariant)
    attn   = softmax(logits) within the bag
    bag    = sum_i attn[i] * emb[i]             [128]
    score  = bag @ w_cls + b_cls                [2]

Distribution: data-parallel over bags. 8 NeuronCores, 8 bags (4096
instances) per core, weights replicated, no cross-core communication.
Each core returns its 8 bags' scores transposed [2, 8]; host stacks.

Layout: the host pre-transposes each core's x shard to x^T [1024, 4096]
and converts it (and the mat-mul weights) to bf16, halving the HBM
traffic — the kernel is DMA-bound — and putting the contraction dim
(d_in) on SBUF partitions directly, avoiding 256 on-chip PE transposes
+ PSUM evacuation per core. Matmuls accumulate in f32 PSUM; biases,
softmax and the bag reduction stay f32 (measured ~7e-4 rel err vs the
f32 reference). Everything on-chip stays transposed (emb^T [128 emb,
512 inst], a^T [64 att, 512 inst], logits [1, 512]) so per-partition
bias APs implement the +b terms and the per-bag softmax is a free-axis
reduce. The softmax skips the max-shift (logits = a @ w_score with a in
(-1,1) tanh-bounded, so exp cannot overflow) and defers 1/denominator
to the [2, 8] classifier epilogue. The bag-weighted sum multiplies
emb^T by the exp row broadcast across partitions via a K=1 matmul with
a ones column. Per-bag emission is software-pipelined (bag b's softmax
tail is emitted after bag b+1's encoder matmuls) so the in-order PE
queue never head-of-line blocks; steady state runs at the ~360 GB/s
HBM roofline (~2.9 us per 1.05 MB bag slab).
"""

import sys

sys.path.insert(0, "/opt/trn_rl_repo")

import numpy as np

N_INST = 32768
N_BAGS = 64
D_IN = 1024
D_EMB = 128
D_ATT = 64
N_CLS = 2

N_CORES = 8
BAGS_PER_CORE = N_BAGS // N_CORES          # 8
INST_PER_BAG = N_INST // N_BAGS            # 512
INST_PER_CORE = N_INST // N_CORES          # 4096
DIN_CHUNKS = D_IN // 128                   # 8
SLAB_SPLIT = 2                             # DMAs per bag slab
CH_PER_DMA = DIN_CHUNKS // SLAB_SPLIT      # 2

_CACHE = {}


def _build():
    import concourse.bacc as bacc
    import concourse.mybir as mybir
    import concourse.tile as tile

    f32 = mybir.dt.float32
    f32r = mybir.dt.float32r
    bf16 = mybir.dt.bfloat16
    AF = mybir.ActivationFunctionType

    nc = bacc.Bacc("TRN2", target_bir_lowering=False, debug=False,
                   enable_asserts=False, num_devices=N_CORES)

    xt = nc.dram_tensor("xt", [D_IN, INST_PER_CORE], bf16, kind="ExternalInput")
    w_enc = nc.dram_tensor("w_enc", [128, D_IN], bf16, kind="ExternalInput")
    b_enc = nc.dram_tensor("b_enc", [D_EMB], f32, kind="ExternalInput")
    w_att = nc.dram_tensor("w_att", [D_EMB, D_ATT], bf16, kind="ExternalInput")
    b_att = nc.dram_tensor("b_att", [D_ATT], f32, kind="ExternalInput")
    w_score = nc.dram_tensor("w_score", [D_ATT], bf16, kind="ExternalInput")
    w_cls = nc.dram_tensor("w_cls", [D_EMB, N_CLS], f32, kind="ExternalInput")
    b_cls = nc.dram_tensor("b_cls", [N_CLS], f32, kind="ExternalInput")
    out = nc.dram_tensor("out", [N_CLS, BAGS_PER_CORE], f32,
                         kind="ExternalOutput")

    with tile.TileContext(nc) as tc:
        with (
            tc.tile_pool(name="const", bufs=1) as const,
            tc.tile_pool(name="xt", bufs=6) as xt_pool,
            tc.tile_pool(name="work", bufs=3) as work,
            tc.tile_pool(name="ps", bufs=2, space="PSUM") as ps,
        ):
            # ---- replicated weights ----
            # host supplies w_enc pre-packed as [128 din-part, chunk*emb]
            wenc_sb = const.tile([128, DIN_CHUNKS, D_EMB], bf16)
            nc.sync.dma_start(
                out=wenc_sb,
                in_=w_enc[:, :].rearrange("p (c e) -> p c e", c=DIN_CHUNKS),
            )
            benc_sb = const.tile([D_EMB, 1], f32)
            nc.scalar.dma_start(
                out=benc_sb, in_=b_enc[:].rearrange("(p one) -> p one", one=1))
            watt_sb = const.tile([D_EMB, D_ATT], bf16)
            nc.scalar.dma_start(out=watt_sb, in_=w_att[:, :])
            batt_sb = const.tile([D_ATT, 1], f32)
            nc.scalar.dma_start(
                out=batt_sb, in_=b_att[:].rearrange("(p one) -> p one", one=1))
            wscore_sb = const.tile([D_ATT, 1], bf16)
            nc.scalar.dma_start(
                out=wscore_sb,
                in_=w_score[:].rearrange("(p one) -> p one", one=1))
            wcls_sb = const.tile([D_EMB, N_CLS], f32)
            nc.scalar.dma_start(out=wcls_sb, in_=w_cls[:, :])
            bcls_sb = const.tile([N_CLS, 1], f32)
            nc.scalar.dma_start(
                out=bcls_sb, in_=b_cls[:].rearrange("(p one) -> p one", one=1))
            ones_row = const.tile([1, 128], bf16)
            nc.vector.memset(ones_row, 1.0)
            ones_f32r = const.tile([1, N_CLS], f32r)
            ones_tmp = const.tile([1, N_CLS], f32)
            nc.vector.memset(ones_tmp, 1.0)
            nc.vector.tensor_copy(ones_f32r, ones_tmp)

            # unnormalized bag embeddings (columns) + softmax denominators
            bag_all = const.tile([D_EMB, BAGS_PER_CORE], f32)
            den_all = const.tile([1, BAGS_PER_CORE], f32)

            xt_re = xt[:, :].rearrange("(c p) i -> p c i", p=128)

            def emit_enc(b):
                i0 = b * INST_PER_BAG
                # split the bag slab into DMAs so the first encoder
                # matmuls start as soon as their chunks land
                parts = []
                for j in range(SLAB_SPLIT):
                    part = xt_pool.tile([128, CH_PER_DMA, INST_PER_BAG], bf16,
                                        tag=f"slab{j}")
                    c0 = j * CH_PER_DMA
                    nc.sync.dma_start(
                        out=part,
                        in_=xt_re[:, c0:c0 + CH_PER_DMA,
                                  i0:i0 + INST_PER_BAG])
                    parts.append(part)

                # emb^T = relu(sum_c w_enc_c.T @ xt_c + b_enc)
                ps_emb = ps.tile([D_EMB, INST_PER_BAG], f32, tag="emb")
                for c in range(DIN_CHUNKS):
                    nc.tensor.matmul(ps_emb[:, :], wenc_sb[:, c, :],
                                     parts[c // CH_PER_DMA][:, c % CH_PER_DMA, :],
                                     start=(c == 0), stop=(c == DIN_CHUNKS - 1))
                embT = work.tile([D_EMB, INST_PER_BAG], bf16, tag="embT")
                nc.scalar.activation(embT, ps_emb, AF.Relu, bias=benc_sb,
                                     scale=1.0)
                return embT

            def emit_tail(b, embT):
                # a^T = tanh(w_att.T @ emb^T + b_att)
                ps_a = ps.tile([D_ATT, INST_PER_BAG], f32, tag="a")
                nc.tensor.matmul(ps_a[:, :], watt_sb[:, :], embT[:, :],
                                 start=True, stop=True)
                aT = work.tile([D_ATT, INST_PER_BAG], bf16, tag="aT")
                nc.scalar.activation(aT, ps_a, AF.Tanh, bias=batt_sb, scale=1.0)

                # logits = w_score.T @ a^T   [1, 512]
                ps_l = ps.tile([1, INST_PER_BAG], f32, tag="logit")
                nc.tensor.matmul(ps_l[:, :], wscore_sb[:, :], aT[:, :],
                                 start=True, stop=True)

                # softmax numerator row + denominator (normalization
                # deferred). No max-shift: logits = a @ w_score with
                # a in (-1,1), so |logits| <= ||w_score||_1 ~ 6 — exp is safe.
                e_row = work.tile([1, INST_PER_BAG], bf16, tag="e_row")
                nc.scalar.activation(e_row, ps_l, AF.Exp, scale=1.0)
                nc.vector.reduce_sum(den_all[0:1, b:b + 1], e_row,
                                     axis=mybir.AxisListType.X)

                # broadcast e row across 128 partitions via K=1 matmul
                ps_bc = ps.tile([D_EMB, INST_PER_BAG], f32, tag="bc")
                nc.tensor.matmul(ps_bc[:, :], ones_row[:, :], e_row[:, :],
                                 start=True, stop=True)

                # unnormalized bag = sum_i emb^T[:, i] * e[i]
                scratch = work.tile([D_EMB, INST_PER_BAG], bf16, tag="scratch")
                nc.vector.tensor_mul(scratch, embT[:, :], ps_bc[:, :])
                nc.vector.reduce_sum(bag_all[:, b:b + 1], scratch,
                                     axis=mybir.AxisListType.X)

            # software pipeline: emit bag b's dependent tail after bag b+1's
            # encoder matmuls so the in-order PE queue never head-of-line
            # blocks on the softmax chain
            def emit_tail_halves(b, embT):
                # the last bag's tail is the serial end-of-kernel chain:
                # split it into two 256-instance halves so the PE/ACT/DVE
                # stages pipeline against each other
                H = INST_PER_BAG // 2
                den_h = work.tile([1, 2], f32, tag="den_h")
                bag_h = work.tile([D_EMB, 2], f32, tag="bag_h")
                for h in range(2):
                    sl = slice(h * H, (h + 1) * H)
                    ps_a = ps.tile([D_ATT, H], f32, tag="a")
                    nc.tensor.matmul(ps_a[:, :], watt_sb[:, :], embT[:, sl],
                                     start=True, stop=True)
                    aT = work.tile([D_ATT, H], bf16, tag="aT")
                    nc.scalar.activation(aT, ps_a, AF.Tanh, bias=batt_sb,
                                         scale=1.0)
                    ps_l = ps.tile([1, H], f32, tag="logit")
                    nc.tensor.matmul(ps_l[:, :], wscore_sb[:, :], aT[:, :],
                                     start=True, stop=True)
                    e_row = work.tile([1, H], bf16, tag="e_row")
                    nc.scalar.activation(e_row, ps_l, AF.Exp, scale=1.0)
                    nc.vector.reduce_sum(den_h[0:1, h:h + 1], e_row,
                                         axis=mybir.AxisListType.X)
                    ps_bc = ps.tile([D_EMB, H], f32, tag="bc")
                    nc.tensor.matmul(ps_bc[:, :], ones_row[:, :], e_row[:, :],
                                     start=True, stop=True)
                    scratch = work.tile([D_EMB, H], bf16, tag="scratch")
                    nc.vector.tensor_mul(scratch, embT[:, sl], ps_bc[:, :])
                    nc.vector.reduce_sum(bag_h[:, h:h + 1], scratch,
                                         axis=mybir.AxisListType.X)
                nc.vector.tensor_add(den_all[0:1, b:b + 1], den_h[0:1, 0:1],
                                     den_h[0:1, 1:2])
                nc.vector.tensor_add(bag_all[:, b:b + 1], bag_h[:, 0:1],
                                     bag_h[:, 1:2])

            prev = None
            for b in range(BAGS_PER_CORE):
                embT = emit_enc(b)
                if prev is not None:
                    emit_tail(b - 1, prev)
                prev = embT
            emit_tail_halves(BAGS_PER_CORE - 1, prev)

            # scores^T = (w_cls.T @ bag_u) * (1/den) + b_cls   [2, 8]
            ps_s = ps.tile([N_CLS, BAGS_PER_CORE], f32, tag="logit")
            nc.tensor.matmul(ps_s[:, :], wcls_sb[:, :], bag_all[:, :],
                             start=True, stop=True)
            rden_row = const.tile([1, BAGS_PER_CORE], f32r)
            with nc.allow_low_precision(reason="1/denom at f32r, ~1e-4 rel"):
                nc.vector.reciprocal(rden_row, den_all)
            ps_r = ps.tile([N_CLS, BAGS_PER_CORE], f32, tag="bc")
            nc.tensor.matmul(ps_r[:, :], ones_f32r[:, :], rden_row[:, :],
                             start=True, stop=True)
            s_u = const.tile([N_CLS, BAGS_PER_CORE], f32)
            nc.scalar.activation(s_u, ps_s[:, :], AF.Copy)
            s_n = const.tile([N_CLS, BAGS_PER_CORE], f32)
            nc.vector.tensor_mul(s_n, s_u, ps_r[:, :])
            scores = const.tile([N_CLS, BAGS_PER_CORE], f32)
            nc.scalar.activation(scores, s_n, AF.Identity, bias=bcls_sb,
                                 scale=1.0)
            nc.scalar.dma_start(out=out[:, :], in_=scores)

    nc.compile()
    return nc


def _numpy_fallback(x, seg, w_enc, b_enc, w_att, b_att, w_score, b_score,
                    w_cls, b_cls):
    emb = np.maximum(x @ w_enc + b_enc, 0.0)
    a = np.tanh(emb @ w_att + b_att)
    logits = a @ w_score + b_score[0]
    out = np.zeros((N_BAGS, N_CLS), dtype=np.float32)
    for bag in range(N_BAGS):
        mask = seg == bag
        lg = logits[mask]
        e = np.exp(lg - lg.max())
        attn = e / e.sum()
        bag_emb = attn @ emb[mask]
        out[bag] = bag_emb @ w_cls + b_cls
    return out


def kernel(**inputs):
    from concourse.bass_utils import run_bass_kernel_spmd

    import ml_dtypes

    x = np.asarray(inputs["x"], dtype=np.float32)
    seg = np.asarray(inputs["seg"], dtype=np.int32)
    w_enc = np.asarray(inputs["w_enc"], dtype=np.float32)
    b_enc = np.asarray(inputs["b_enc"], dtype=np.float32)
    w_att = np.asarray(inputs["w_att"], dtype=np.float32)
    b_att = np.asarray(inputs["b_att"], dtype=np.float32)
    w_score = np.asarray(inputs["w_score"], dtype=np.float32)
    b_score = np.asarray(inputs["b_score"], dtype=np.float32)
    w_cls = np.asarray(inputs["w_cls"], dtype=np.float32)
    b_cls = np.asarray(inputs["b_cls"], dtype=np.float32)

    expected_seg = np.repeat(np.arange(N_BAGS, dtype=np.int32), INST_PER_BAG)
    if not np.array_equal(seg, expected_seg):
        # Layout differs from the balanced bags this kernel is built for.
        return _numpy_fallback(x, seg, w_enc, b_enc, w_att, b_att, w_score,
                               b_score, w_cls, b_cls)

    if "nc" not in _CACHE:
        _CACHE["nc"] = _build()
    nc = _CACHE["nc"]

    shared = {
        "w_enc": np.ascontiguousarray(
            w_enc.astype(ml_dtypes.bfloat16).reshape(DIN_CHUNKS, 128, D_EMB)
            .transpose(1, 0, 2).reshape(128, D_IN)),
        "b_enc": b_enc,
        "w_att": w_att.astype(ml_dtypes.bfloat16), "b_att": b_att,
        "w_score": w_score.astype(ml_dtypes.bfloat16),
        "w_cls": w_cls, "b_cls": b_cls,
    }
    in_maps = []
    for c in range(N_CORES):
        xs = x[c * INST_PER_CORE:(c + 1) * INST_PER_CORE]
        in_maps.append(
            {"xt": np.ascontiguousarray(xs.T).astype(ml_dtypes.bfloat16),
             **shared})

    res = run_bass_kernel_spmd(nc, in_maps, core_ids=list(range(N_CORES)))
    return np.concatenate(
        [res.results[c]["out"].T for c in range(N_CORES)], axis=0)



# revision 2
# speedup vs baseline: 1.3847x; 1.3847x over previous
"""AttentionMIL Trainium2 kernel (v2: fp8 encoder + restructured tail).

Math (per bag of 512 instances):
    emb    = relu(x @ w_enc + b_enc)            [512, 128]
    a      = tanh(emb @ w_att + b_att)          [512, 64]
    logits = a @ w_score (+ b_score, dropped: softmax shift-invariant)
    attn   = softmax(logits) within the bag
    bag    = sum_i attn[i] * emb[i]             [128]
    score  = bag @ w_cls + b_cls                [2]

Distribution: data-parallel over bags. 8 NeuronCores, 8 bags (4096
instances) per core, weights replicated, no cross-core communication.
Each core returns its 8 bags' scores transposed [2, 8]; host stacks.

v2 design, driven by the v1 trace (PE cold at 1.2 GHz for the first
18.7 us, 44 us PE busy, 24 tail matmuls at full 512-cycle cost, DVE
doing [128,512] broadcasts):

- x and w_enc are quantized to fp8 e4m3 on the host (rel err 5.9e-3 vs
  the f32 reference, gate is 2e-2). Halves HBM traffic to ~4.3 MB/core
  (~12 us at 358 GB/s) and enables DoubleRow matmuls: each encoder MM
  contracts TWO 128-row K-chunks (2 fp8 weights/PE cell), so a bag's
  encoder is 4 MMs instead of 8.
- A warm-up burst of dummy N=128 matmuls at t=0 (overlapping the first
  DMAs) gets the PE HAM clock gate to K=8/8 (2.4 GHz) before the real
  matmuls start; v1 ran its first third at half clock.
- The per-bag tail never touches [128, 512] tensors again: the
  classifier is contracted EARLY (Y = w_cls^T @ embT, a [2,512] strip
  col-tiled to run concurrently with the [64,512] attention MM), the
  per-instance logit row is computed twice into a [2,512] PSUM strip so
  exp lands partition-aligned with Y, and the softmax reduction is a
  single fused DVE scalar_tensor_tensor (prod = Y * e2, accum_out =
  row-sum) per bag. Denominators fall out of the exp activation's
  accum_out for free. Per-bag engine cost: PE 4 DR + att&Y + logits2,
  ACT tanh + exp, DVE relu(+bias) + one fused mul-reduce.
- Bag slabs are host-packed so each partition's data is one contiguous
  2 KB line per half-slab DMA; one DMA per half-bag on the sync HWDGE
  queue.
- relu (+b_enc, via tensor_scalar add/max) runs on DVE, balancing ACT
  (tanh+exp) at ~1.5 us/bag each.
"""

import sys

sys.path.insert(0, "/opt/trn_rl_repo")

import numpy as np

N_INST = 32768
N_BAGS = 64
D_IN = 1024
D_EMB = 128
D_ATT = 64
N_CLS = 2

N_CORES = 8
BAGS_PER_CORE = N_BAGS // N_CORES          # 8
INST_PER_BAG = N_INST // N_BAGS            # 512
INST_PER_CORE = N_INST // N_CORES          # 4096
DIN_CHUNKS = D_IN // 128                   # 8
N_WARMUP = 20                              # PE HAM warm-up matmuls

_CACHE = {}


def _build():
    import concourse.bacc as bacc
    import concourse.mybir as mybir
    import concourse.tile as tile

    f32 = mybir.dt.float32
    f32r = mybir.dt.float32r
    bf16 = mybir.dt.bfloat16
    fp8 = mybir.dt.float8e4
    AF = mybir.ActivationFunctionType
    ALU = mybir.AluOpType
    DR = mybir.MatmulPerfMode.DoubleRow

    nc = bacc.Bacc("TRN2", target_bir_lowering=False, debug=False,
                   enable_asserts=False, num_devices=N_CORES)

    # x packed [bag, p, chunk, inst]; row c*128+p of x^T lives at [:, p, c, :]
    xt = nc.dram_tensor("xt", [BAGS_PER_CORE, 128, DIN_CHUNKS, INST_PER_BAG],
                        fp8, kind="ExternalInput")
    # w_enc packed [p, chunk, emb] with the same (c, p) row mapping
    w_enc = nc.dram_tensor("w_enc", [128, DIN_CHUNKS, D_EMB], fp8,
                           kind="ExternalInput")
    # [w_att | w_cls | w_score2]: w_score2 rows 64:128 hold w_score twice
    wtail = nc.dram_tensor("wtail", [128, D_ATT + N_CLS + N_CLS], bf16,
                           kind="ExternalInput")
    # col 0 = b_enc, col 1 = b_att (rows 64:128), col 2 rows 0:2 = b_cls
    btail = nc.dram_tensor("btail", [128, 3], f32, kind="ExternalInput")
    out = nc.dram_tensor("out", [N_CLS, BAGS_PER_CORE], f32,
                         kind="ExternalOutput")

    with tile.TileContext(nc) as tc:
        with (
            tc.tile_pool(name="const", bufs=1) as const,
            tc.tile_pool(name="slab", bufs=6) as slab_pool,
            tc.tile_pool(name="embp", bufs=3) as emb_pool,
            tc.tile_pool(name="atp", bufs=3) as at_pool,
            tc.tile_pool(name="e2p", bufs=2) as e2_pool,
            tc.tile_pool(name="prodp", bufs=2) as prod_pool,
            tc.tile_pool(name="ps_wu", bufs=1, space="PSUM") as ps_wu,
            tc.tile_pool(name="ps_emb", bufs=2, space="PSUM") as ps_emb_pool,
            tc.tile_pool(name="ps_ay", bufs=3, space="PSUM") as ps_ay_pool,
            tc.tile_pool(name="ps_l", bufs=2, space="PSUM") as ps_l_pool,
        ):
            # ---- warm-up operand (zeros; only PE activity matters) ----
            wu_rhs = const.tile([128, 128], fp8)
            nc.vector.memset(wu_rhs, 0.0)

            # ---- replicated weights (sync HWDGE queue, encoder first) ----
            wenc_sb = const.tile([128, DIN_CHUNKS, D_EMB], fp8)
            nc.sync.dma_start(out=wenc_sb, in_=w_enc[:, :, :])

            # bag 0's slab ahead of the small tail weights
            halves = {}
            H_CH = DIN_CHUNKS // 2

            def emit_slab(b):
                hs = []
                for h in range(2):
                    t = slab_pool.tile([128, H_CH, INST_PER_BAG], fp8,
                                       tag=f"slab{h}")
                    nc.sync.dma_start(
                        out=t, in_=xt[b, :, h * H_CH:(h + 1) * H_CH, :])
                    hs.append(t)
                halves[b] = hs

            emit_slab(0)

            wtail_sb = const.tile([128, D_ATT + 2 * N_CLS], bf16)
            nc.sync.dma_start(out=wtail_sb, in_=wtail[:, :])
            btail_sb = const.tile([128, 3], f32)
            nc.sync.dma_start(out=btail_sb, in_=btail[:, :])

            watt = wtail_sb[:, 0:D_ATT]
            wcls = wtail_sb[:, D_ATT:D_ATT + N_CLS]
            ws2 = wtail_sb[64:128, D_ATT + N_CLS:D_ATT + 2 * N_CLS]
            benc = btail_sb[:, 0:1]
            batt = btail_sb[64:128, 1:2]
            bcls = btail_sb[0:2, 2:3]

            # ---- PE warm-up: release the HAM clock gate before real MMs ----
            wu_ps = ps_wu.tile([128, 128], f32)
            for _ in range(N_WARMUP):
                nc.tensor.matmul(wu_ps[:, :], wu_rhs[:, :], wu_rhs[:, :],
                                 start=True, stop=True)

            den_all = const.tile([N_CLS, BAGS_PER_CORE], f32)
            sc_all = const.tile([N_CLS, BAGS_PER_CORE], f32)

            embT = {}
            aT = {}
            ps_ay = {}

            def emit_enc(b):
                hs = halves[b]
                ps_emb = ps_emb_pool.tile([D_EMB, INST_PER_BAG], f32,
                                          tag="emb")
                for j in range(DIN_CHUNKS // 2):
                    h, lj = divmod(j, H_CH // 2)
                    nc.tensor.matmul(
                        ps_emb[:, :],
                        wenc_sb[:, 2 * j:2 * j + 2, :],
                        hs[h][:, 2 * lj:2 * lj + 2, :],
                        start=(j == 0), stop=(j == DIN_CHUNKS // 2 - 1),
                        perf_mode=DR)
                t = emb_pool.tile([D_EMB, INST_PER_BAG], bf16, tag="embT")
                # embT = max(ps_emb + b_enc, 0) in one DVE op
                nc.vector.tensor_scalar(t, ps_emb[:, :], benc, 0.0,
                                        op0=ALU.add, op1=ALU.max)
                embT[b] = t

            def emit_att_y(b):
                # one PSUM bank: rows 0:2 = Y = w_cls^T emb^T (col strip 0),
                # rows 64:128 = pre-tanh attention (col strip 64) — the two
                # matmuls run concurrently on disjoint PE column groups
                ps = ps_ay_pool.tile([128, INST_PER_BAG], f32, tag="ay")
                nc.tensor.matmul(ps[0:2, :], wcls, embT[b][:, :],
                                 start=True, stop=True)
                nc.tensor.matmul(ps[64:128, :], watt, embT[b][:, :],
                                 start=True, stop=True)
                t = at_pool.tile([128, INST_PER_BAG], bf16, tag="aT")
                nc.scalar.activation(t[64:128, :], ps[64:128, :], AF.Tanh,
                                     bias=batt, scale=1.0)
                ps_ay[b] = ps
                aT[b] = t

            def emit_logexp(b):
                # duplicated-row logits so exp lands on partitions 0:2,
                # aligned with Y for the fused DVE reduction
                ps_l = ps_l_pool.tile([N_CLS, INST_PER_BAG], f32, tag="pl")
                nc.tensor.matmul(ps_l[:, :], ws2, aT[b][64:128, :],
                                 start=True, stop=True)
                e2 = e2_pool.tile([N_CLS, INST_PER_BAG], bf16, tag="e2")
                # no max-shift: |logits| <= ||w_score||_1 ~ 6, exp is safe
                nc.scalar.activation(e2, ps_l[:, :], AF.Exp, scale=1.0,
                                     accum_out=den_all[:, b:b + 1])
                prod = prod_pool.tile([N_CLS, INST_PER_BAG], f32, tag="prod")
                # prod = Y * e2; accum_out = per-bag unnormalized scores
                nc.vector.scalar_tensor_tensor(
                    prod, ps_ay[b][0:2, :], 1.0, e2,
                    op0=ALU.mult, op1=ALU.mult,
                    accum_out=sc_all[:, b:b + 1])
                del ps_ay[b], aT[b]

            # software pipeline: enc(b) | attY(b-1) | logexp(b-2) so the
            # in-order PE queue never waits on an ACT result
            for b in range(BAGS_PER_CORE):
                if b > 0:
                    emit_slab(b)
                emit_enc(b)
                if b >= 1:
                    emit_att_y(b - 1)
                if b >= 2:
                    emit_logexp(b - 2)
            emit_att_y(BAGS_PER_CORE - 1)
            emit_logexp(BAGS_PER_CORE - 2)
            emit_logexp(BAGS_PER_CORE - 1)

            # ---- epilogue: scores = sc_all / den + b_cls  [2, 8] ----
            rden_r = const.tile([N_CLS, BAGS_PER_CORE], f32r)
            with nc.allow_low_precision(reason="1/denom at f32r, ~1e-4 rel"):
                nc.vector.reciprocal(rden_r, den_all)
            rden = const.tile([N_CLS, BAGS_PER_CORE], f32)
            nc.vector.tensor_copy(rden, rden_r)
            s_n = const.tile([N_CLS, BAGS_PER_CORE], f32)
            nc.vector.tensor_mul(s_n, sc_all, rden)
            scores = const.tile([N_CLS, BAGS_PER_CORE], f32)
            nc.scalar.activation(scores, s_n, AF.Identity, bias=bcls,
                                 scale=1.0)
            nc.sync.dma_start(out=out[:, :], in_=scores)

    nc.compile()
    return nc


def _numpy_fallback(x, seg, w_enc, b_enc, w_att, b_att, w_score, b_score,
                    w_cls, b_cls):
    emb = np.maximum(x @ w_enc + b_enc, 0.0)
    a = np.tanh(emb @ w_att + b_att)
    logits = a @ w_score + b_score[0]
    out = np.zeros((N_BAGS, N_CLS), dtype=np.float32)
    for bag in range(N_BAGS):
        mask = seg == bag
        lg = logits[mask]
        e = np.exp(lg - lg.max())
        attn = e / e.sum()
        bag_emb = attn @ emb[mask]
        out[bag] = bag_emb @ w_cls + b_cls
    return out


def make_in_maps(inputs):
    import ml_dtypes

    e4 = ml_dtypes.float8_e4m3fn
    bf16 = ml_dtypes.bfloat16

    x = np.asarray(inputs["x"], dtype=np.float32)
    w_enc = np.asarray(inputs["w_enc"], dtype=np.float32)
    w_att = np.asarray(inputs["w_att"], dtype=np.float32)
    w_score = np.asarray(inputs["w_score"], dtype=np.float32)
    w_cls = np.asarray(inputs["w_cls"], dtype=np.float32)

    wenc_p = np.ascontiguousarray(
        w_enc.reshape(DIN_CHUNKS, 128, D_EMB).transpose(1, 0, 2)).astype(e4)

    wtail = np.zeros((128, D_ATT + 2 * N_CLS), dtype=bf16)
    wtail[:, 0:D_ATT] = w_att.astype(bf16)
    wtail[:, D_ATT:D_ATT + N_CLS] = w_cls.astype(bf16)
    wtail[64:128, D_ATT + N_CLS] = w_score.astype(bf16)
    wtail[64:128, D_ATT + N_CLS + 1] = w_score.astype(bf16)

    btail = np.zeros((128, 3), dtype=np.float32)
    btail[:, 0] = np.asarray(inputs["b_enc"], dtype=np.float32)
    btail[64:128, 1] = np.asarray(inputs["b_att"], dtype=np.float32)
    btail[0:2, 2] = np.asarray(inputs["b_cls"], dtype=np.float32)

    shared = {"w_enc": wenc_p, "wtail": wtail, "btail": btail}

    xq = x.astype(e4)
    in_maps = []
    for c in range(N_CORES):
        xs = xq[c * INST_PER_CORE:(c + 1) * INST_PER_CORE]
        # [bag, inst, chunk, p] -> [bag, p, chunk, inst]
        xp = np.ascontiguousarray(
            xs.reshape(BAGS_PER_CORE, INST_PER_BAG, DIN_CHUNKS, 128)
            .transpose(0, 3, 2, 1))
        in_maps.append({"xt": xp, **shared})
    return in_maps


def kernel(**inputs):
    from concourse.bass_utils import run_bass_kernel_spmd

    x = np.asarray(inputs["x"], dtype=np.float32)
    seg = np.asarray(inputs["seg"], dtype=np.int32)

    expected_seg = np.repeat(np.arange(N_BAGS, dtype=np.int32), INST_PER_BAG)
    if not np.array_equal(seg, expected_seg):
        # Layout differs from the balanced bags this kernel is built for.
        return _numpy_fallback(
            x, seg,
            *(np.asarray(inputs[k], dtype=np.float32) for k in
              ("w_enc", "b_enc", "w_att", "b_att", "w_score", "b_score",
               "w_cls", "b_cls")))

    if "nc" not in _CACHE:
        _CACHE["nc"] = _build()
    nc = _CACHE["nc"]

    in_maps = make_in_maps(inputs)
    res = run_bass_kernel_spmd(nc, in_maps, core_ids=list(range(N_CORES)))
    return np.concatenate(
        [res.results[c]["out"].T for c in range(N_CORES)], axis=0)
